# revision 17
# baseline (speedup 1.0000x reference)
"""Swin-style windowed cross-attention on 8 Trainium2 cores (Bass/Tile).

Data-parallel over the window dim B_: each core gets B_/8 windows.
Per-core pipeline (matmuls bf16, fp32 PSUM):
  - feature-major Q_T = (q_w*scale).T-proj of x.T            (PE)
  - feature-major K_T and natural-layout V from mem.T        (PE)
  - per window w:
      bdQ   = block-diag spread of Q_T column (8 rect DMAs)
      S.T   = K_T.T @ bdQ   2 full-array matmuls -> (keys 128, (h,n) 512)
      P_u   = exp(S.T)                                       (ACT)
      P_b   = P_u * exp(bias).T  [resident tile]             (DVE)
      D     = blockdiag_ones.T @ P_b  (bcast key-sums)       (PE)
      P_n   = P_b * recip(D)                                 (DVE)
      p1/v1 = partition-rebase DMAs (t=1 rows -> base 0)
      O.T   = 16 small matmuls, 4 col-position chains into 4 PSUM banks
      FINAL = O.T.T @ proj_w.T  (natural layout, fp32)       (PE)
All same-PSUM-bank matmuls are position-serialized (full-array or same
tile_position) - concurrent tile-positioned matmuls must not share a bank.
"""
import os
import sys

sys.path.insert(0, "/opt/trn_rl_repo")

import numpy as np
import ml_dtypes

PH, PW = 8, 8
H = 8
N = 64
C = 256
HD = 32
T = 2
N_CORES = 8
B_FULL = 2048

BF16 = ml_dtypes.bfloat16


def _relative_position_index(ph, pw):
    coords = np.stack(np.meshgrid(np.arange(ph), np.arange(pw), indexing="ij"))
    flat = coords.reshape(2, -1)
    rel = flat[:, :, None] - flat[:, None, :]
    rel = rel.transpose(1, 2, 0).copy()
    rel[:, :, 0] += ph - 1
    rel[:, :, 1] += pw - 1
    rel[:, :, 0] *= 2 * pw - 1
    return rel.sum(-1)


REL_IDX = _relative_position_index(PH, PW)

_CACHE = {}


def build_bass(b):
    """Build the single-core Bass program for b windows."""
    import concourse.bass as bass
    import concourse.tile as tile
    from concourse import bacc, mybir
    from contextlib import ExitStack

    bf = mybir.dt.bfloat16
    f32 = mybir.dt.float32
    EXP = mybir.ActivationFunctionType.Exp

    QB = 8   # windows per x/q block
    MB = 4   # windows per mem/k/v block
    NBDQ = 3
    assert b % QB == 0 and b % MB == 0

    rows_x = b * N
    rows_m = b * T * N

    CW = 2 * C + 4 * C + 2 * C + H * N + T * N  # 2688
    nc = bacc.Bacc("TRN2", target_bir_lowering=False)
    xT = nc.declare_dram_parameter("xT", [C, rows_x], bf, isOutput=False)
    memT = nc.declare_dram_parameter("memT", [C, rows_m], bf, isOutput=False)
    consts = nc.declare_dram_parameter("consts", [128, CW], bf, isOutput=False)
    out = nc.declare_dram_parameter("out", [rows_m, C], f32, isOutput=True)

    with tile.TileContext(nc) as tc, ExitStack() as ctx:
        const = ctx.enter_context(tc.tile_pool(name="const", bufs=1))
        cw_sb = const.tile([128, CW], bf)
        nc.sync.dma_start(out=cw_sb[:], in_=consts[:, :])
        qw_sb = cw_sb[:, 0:2 * C]                    # col = ct*256 + o
        kvw_sb = cw_sb[:, 2 * C:6 * C]               # col = ct*512 + o
        pw_sb = cw_sb[:, 6 * C:8 * C]                # col = f*256 + o
        ebt_sb = cw_sb[:, 8 * C:8 * C + H * N]
        bdo_sb = cw_sb[:, 8 * C + H * N:8 * C + H * N + T * N]

        xt_pool = ctx.enter_context(tc.tile_pool(name="xt", bufs=3))
        qt_pool = ctx.enter_context(tc.tile_pool(name="qt", bufs=2))
        mt_pool = ctx.enter_context(tc.tile_pool(name="mt", bufs=3))
        kt_pool = ctx.enter_context(tc.tile_pool(name="kt", bufs=2))
        v_pool = ctx.enter_context(tc.tile_pool(name="v", bufs=MB + 2))
        v1_pool = ctx.enter_context(tc.tile_pool(name="v1", bufs=MB + 2))
        bdq_pool = ctx.enter_context(tc.tile_pool(name="bdq", bufs=1))
        p_pool = ctx.enter_context(tc.tile_pool(name="p", bufs=3))
        ot_pool = ctx.enter_context(tc.tile_pool(name="ot", bufs=3))
        f_pool = ctx.enter_context(tc.tile_pool(name="f", bufs=4))

        ps_sd = ctx.enter_context(tc.tile_pool(name="ps_sd", bufs=2, space="PSUM"))
        ps_misc = ctx.enter_context(tc.tile_pool(name="ps_misc", bufs=2, space="PSUM"))
        ps_pv = ctx.enter_context(tc.tile_pool(name="ps_pv", bufs=1, space="PSUM"))

        # fixed rotating block-diag-Q tiles; zeros persist outside the
        # diagonal blocks, which are rewritten every window
        bdq_tiles = []
        for i in range(NBDQ):
            t_ = bdq_pool.tile([128, 2, 4 * N], bf, tag=f"bdq{i}")
            nc.vector.memset(t_[:], 0.0)
            bdq_tiles.append(t_)

        xt = qt = mt = kt = None
        v_tiles = [None] * MB
        v1_tiles = [None] * MB

        for w in range(b):
            qb_i, w_q = divmod(w, QB)
            mb_i, w_m = divmod(w, MB)

            if w_q == 0:
                xt = xt_pool.tile([128, 2, QB * N], bf, tag="xt")
                nc.sync.dma_start(
                    out=xt[:],
                    in_=xT.rearrange("(ct p) n -> p ct n", p=128)[
                        :, :, qb_i * QB * N:(qb_i + 1) * QB * N])
                qt = qt_pool.tile([128, 2, QB * N], bf, tag="qt")
                for oh in range(2):
                    q_ps = ps_misc.tile([128, QB * N], f32, tag="misc")
                    for ct in range(2):
                        nc.tensor.matmul(
                            q_ps[:],
                            lhsT=qw_sb[:, ct * C + oh * 128: ct * C + (oh + 1) * 128],
                            rhs=xt[:, ct, :],
                            start=(ct == 0), stop=(ct == 1))
                    nc.scalar.copy(qt[:, oh, :], q_ps[:])

            if w_m == 0:
                mt = mt_pool.tile([128, 2, MB * T * N], bf, tag="mt")
                nc.sync.dma_start(
                    out=mt[:],
                    in_=memT.rearrange("(ct p) n -> p ct n", p=128)[
                        :, :, mb_i * MB * T * N:(mb_i + 1) * MB * T * N])
                kt = kt_pool.tile([128, 2, MB * T * N], bf, tag="kt")
                for oh in range(2):
                    k_ps = ps_misc.tile([128, MB * T * N], f32, tag="misc")
                    for ct in range(2):
                        nc.tensor.matmul(
                            k_ps[:],
                            lhsT=kvw_sb[:, ct * 2 * C + oh * 128:
                                        ct * 2 * C + (oh + 1) * 128],
                            rhs=mt[:, ct, :],
                            start=(ct == 0), stop=(ct == 1))
                    nc.scalar.copy(kt[:, oh, :], k_ps[:])
                for wv in range(MB):
                    v_ps = ps_misc.tile([128, C], f32, tag="misc")
                    for ct in range(2):
                        nc.tensor.matmul(
                            v_ps[:],
                            lhsT=mt[:, ct, wv * 128:(wv + 1) * 128],
                            rhs=kvw_sb[:, ct * 2 * C + 256: ct * 2 * C + 512],
                            start=(ct == 0), stop=(ct == 1))
                    v_sb = v_pool.tile([128, C], bf, tag="v_sb")
                    nc.scalar.copy(v_sb[:], v_ps[:])
                    v_tiles[wv] = v_sb
                    v1_sb = v1_pool.tile([64, C], bf, tag="v1_sb")
                    nc.sync.dma_start(out=v1_sb[:], in_=v_sb[64:128, :])
                    v1_tiles[wv] = v1_sb

            # --- block-diag Q for this window: (128, f, (hh, n) 256) ---
            bdq = bdq_tiles[w % NBDQ]
            for f in range(2):
                for hh in range(4):
                    nc.sync.dma_start(
                        out=bdq[32 * hh:32 * (hh + 1), f, hh * N:(hh + 1) * N],
                        in_=qt[32 * hh:32 * (hh + 1), f, w_q * N:(w_q + 1) * N])

            # --- S.T: (keys (t,m) 128, (h, n) 512), 2 full-array matmuls ---
            s_ps = ps_sd.tile([128, H * N], f32, tag="sd")
            for f in range(2):
                nc.tensor.matmul(
                    s_ps[:, f * 4 * N:(f + 1) * 4 * N],
                    lhsT=kt[:, f, w_m * T * N:(w_m + 1) * T * N],
                    rhs=bdq[:, f, :],
                    start=True, stop=True)

            p_u = p_pool.tile([128, H * N], bf, tag="pu")
            nc.scalar.activation(p_u[:], s_ps[:], EXP)
            p_b = p_pool.tile([128, H * N], bf, tag="pb")
            nc.vector.tensor_mul(p_b[:], p_u[:], ebt_sb)

            d_ps = ps_sd.tile([128, H * N], f32, tag="sd")
            nc.tensor.matmul(d_ps[:], lhsT=bdo_sb, rhs=p_b[:],
                             start=True, stop=True)
            r_sb = p_pool.tile([128, H * N], bf, tag="r")
            with nc.allow_low_precision(reason="softmax recip in bf16 is fine"):
                nc.vector.reciprocal(r_sb[:], d_ps[:])
            p_n = p_pool.tile([128, H * N], bf, tag="pn")
            nc.vector.tensor_mul(p_n[:], p_b[:], r_sb[:])
            p1 = p_pool.tile([64, H * N], bf, tag="p1")
            nc.sync.dma_start(out=p1[:], in_=p_n[64:128, :])

            # --- PV: O.T rows (hh*32+d), cols (hg, t, n); 4 banks by hh ---
            o_banks = [ps_pv.tile([128, 2 * T * N], f32, tag=f"pv{hh}",
                                  name=f"opv{hh}") for hh in range(4)]
            v_sb = v_tiles[w_m]
            v1_sb = v1_tiles[w_m]
            for t in range(T):
                for hg in range(2):
                    for hh in range(4):
                        h = hg * 4 + hh
                        lhs = (v_sb if t == 0 else v1_sb)
                        nc.tensor.matmul(
                            o_banks[hh][32 * hh:32 * (hh + 1),
                                        (hg * 2 + t) * N:(hg * 2 + t + 1) * N],
                            lhsT=lhs[0:64, hg * 128 + hh * 32:
                                     hg * 128 + (hh + 1) * 32],
                            rhs=(p_n if t == 0 else p1)[0:64, h * N:(h + 1) * N],
                            start=True, stop=True, tile_position=(0, 32 * hh))

            ot = ot_pool.tile([128, 2 * T * N], bf, tag="ot")
            for hh in range(4):
                nc.vector.tensor_copy(ot[32 * hh:32 * (hh + 1), :],
                                      o_banks[hh][32 * hh:32 * (hh + 1), :])

            # --- output projection: FINAL ((t, n) 128, o 256) fp32 ---
            f_ps = ps_misc.tile([128, C], f32, tag="misc")
            for f in range(2):
                nc.tensor.matmul(
                    f_ps[:],
                    lhsT=ot[:, f * T * N:(f + 1) * T * N],
                    rhs=pw_sb[:, f * C:(f + 1) * C],
                    start=(f == 0), stop=(f == 1))
            f_sb = f_pool.tile([128, C], f32, tag="f_sb")
            nc.vector.tensor_copy(f_sb[:], f_ps[:])
            nc.sync.dma_start(out=out[w * T * N:(w + 1) * T * N, :], in_=f_sb[:])

    nc.compile()
    return nc


def _host_prep(x, memory, q_w, kv_w, proj_w, rpb_table):
    scale = np.float32(HD ** -0.5)
    qwT = (q_w.astype(np.float32) * scale).T.astype(np.float32)   # (c, o)
    kvwT = kv_w.astype(np.float32).T.astype(np.float32)           # (c, o)
    pwT = proj_w.astype(np.float32).T.astype(np.float32)          # (c, o)

    bias = rpb_table[REL_IDX.reshape(-1)].reshape(N, N, H)  # (n, m, h)
    ebt = np.empty((T * N, H * N), np.float32)
    eb = np.exp(bias.astype(np.float32))  # (n, m, h)
    ebt_1 = np.ascontiguousarray(eb.transpose(1, 2, 0)).reshape(N, H * N)
    ebt[:N] = ebt_1
    ebt[N:] = ebt_1

    bdo = np.zeros((T * N, T * N), np.float32)
    bdo[:N, :N] = 1.0
    bdo[N:, N:] = 1.0

    CW = 2 * C + 4 * C + 2 * C + H * N + T * N
    consts = np.zeros((128, CW), np.float32)
    consts[:, 0:2 * C] = qwT.reshape(2, 128, C).transpose(1, 0, 2).reshape(128, 2 * C)
    consts[:, 2 * C:6 * C] = kvwT.reshape(2, 128, 2 * C).transpose(1, 0, 2).reshape(128, 4 * C)
    consts[:, 6 * C:8 * C] = pwT.reshape(2, 128, C).transpose(1, 0, 2).reshape(128, 2 * C)
    consts[:, 8 * C:8 * C + H * N] = ebt
    consts[:, 8 * C + H * N:] = bdo
    return consts.astype(BF16)


def kernel(x, memory, q_w, q_b, kv_w, kv_b, proj_w, proj_b, rpb_table):
    x = np.asarray(x)
    memory = np.asarray(memory)
    q_w = np.asarray(q_w, np.float32)
    q_b = np.asarray(q_b, np.float32)
    kv_w = np.asarray(kv_w, np.float32)
    kv_b = np.asarray(kv_b, np.float32)
    proj_w = np.asarray(proj_w, np.float32)
    proj_b = np.asarray(proj_b, np.float32)
    rpb_table = np.asarray(rpb_table, np.float32)

    if np.any(q_b) or np.any(kv_b):
        raise NotImplementedError("nonzero q_b/kv_b not supported")

    B_ = x.shape[0]
    b = B_ // N_CORES
    consts = _host_prep(x, memory, q_w, kv_w, proj_w, rpb_table)

    key = ("prog", b)
    if key not in _CACHE:
        _CACHE[key] = build_bass(b)
    nc = _CACHE[key]

    in_maps = []
    for i in range(N_CORES):
        xs = x[i * b:(i + 1) * b].reshape(b * N, C)
        ms = memory[i * b * T:(i + 1) * b * T].reshape(b * T * N, C)
        in_maps.append({
            "xT": np.ascontiguousarray(xs.T).astype(BF16),
            "memT": np.ascontiguousarray(ms.T).astype(BF16),
            "consts": consts,
        })

    from concourse.bass_utils import run_bass_kernel_spmd
    res = run_bass_kernel_spmd(nc, in_maps, list(range(N_CORES)))

    outs = [np.asarray(res.results[i]["out"], np.float32) for i in range(N_CORES)]
    full = np.concatenate(outs, axis=0)  # (B_*T*N, C) in (w, t, n) order
    full = full.reshape(B_, T, N, C)
    if np.any(proj_b):
        full = full + proj_b
    return full.astype(np.float32)
